# revision 41
# baseline (speedup 1.0000x reference)
"""Trainium2 Bass kernel for a minimal Mamba layer (B=2, L=2048, d_model=1024,
d_inner=2048, d_state=16, d_conv=4, dt_rank=64) on 8 NeuronCores.

Sharding: core = (batch, d_inner-quarter).  Cores 0-3 handle batch 0, cores
4-7 batch 1; within a batch group each core owns 512 d_inner channels.

Two SPMD kernels with a tiny host exchange between them:
  A: in_proj (own rows) + causal depthwise conv (as 4 PSUM-accumulated
     diagonal matmuls) + silu + x_proj partial (own-channel contraction).
  host: sum the 4 partial dbc's per batch (96x2048 each), build broadcast
     tiles for B/C rows.
  B: dt_proj + softplus, then per (state, ch-block): dA = exp(A*delta) on
     ScalarE, Bu on VectorE, the SSM recurrence via the hardware
     tensor_tensor_scan, y accumulation, gating, out_proj partial.
  host: sum the 4 partial outputs per batch.
"""

import sys

if "/opt/trn_rl_repo" not in sys.path:
    sys.path.insert(0, "/opt/trn_rl_repo")

import numpy as np
import ml_dtypes

import concourse.bass as bass
from concourse import bacc, mybir
from concourse.bass_utils import run_bass_kernel_spmd
from concourse.tile import TileContext

F32 = mybir.dt.float32
BF16 = mybir.dt.bfloat16
AF = mybir.ActivationFunctionType
OP = mybir.AluOpType

D_MODEL = 1024
D_STATE = 16
D_CONV = 4
D_INNER = 2048
DT_RANK = 64
B = 2
L = 2048
NCORES = 8
CH = D_INNER // 4          # 512 channels per core
NCB = CH // 128            # 4 channel blocks of 128
NT = L // 512              # 4 token tiles of 512
KM = D_MODEL // 128        # 8 k tiles for in_proj

_CACHE = {}


def _build_a():
    nc = bacc.Bacc("TRN2", target_bir_lowering=False, debug=False,
                   num_devices=NCORES)
    xT = nc.dram_tensor("xT", [D_MODEL, L], BF16, kind="ExternalInput").ap()
    w1t = nc.dram_tensor("w1t", [D_MODEL, 2 * CH], BF16, kind="ExternalInput").ap()
    convdiag = nc.dram_tensor("convdiag", [D_CONV * NCB * 128, 128], BF16,
                              kind="ExternalInput").ap()
    convb = nc.dram_tensor("convb", [128, NCB], F32, kind="ExternalInput").ap()
    wxpT = nc.dram_tensor("wxpT", [CH, 96], BF16, kind="ExternalInput").ap()

    xc_out = nc.dram_tensor("xc", [CH, L], BF16, kind="ExternalOutput").ap()
    sres_out = nc.dram_tensor("sres", [CH, L], BF16, kind="ExternalOutput").ap()
    dbc_out = nc.dram_tensor("dbc", [96, L], F32, kind="ExternalOutput").ap()

    with TileContext(nc) as tc:
        with (
            tc.tile_pool(name="const", bufs=1) as const,
            tc.tile_pool(name="psum", bufs=4, space="PSUM") as psum,
            tc.tile_pool(name="work", bufs=3) as work,
        ):
            xT_t, w1_t = [], []
            for k in range(KM):
                t = const.tile([128, L], BF16, tag=f"xT{k}")
                nc.sync.dma_start(out=t[:], in_=xT[k * 128:(k + 1) * 128, :])
                xT_t.append(t)
                t = const.tile([128, 2 * CH], BF16, tag=f"w1{k}", name=f"w1{k}")
                nc.sync.dma_start(out=t[:], in_=w1t[k * 128:(k + 1) * 128, :])
                w1_t.append(t)
            cdiag = []
            for tap in range(D_CONV):
                row = []
                for cb in range(NCB):
                    t = const.tile([128, 128], BF16, tag=f"cd{tap}_{cb}")
                    off = (tap * NCB + cb) * 128
                    nc.sync.dma_start(out=t[:], in_=convdiag[off:off + 128, :])
                    row.append(t)
                cdiag.append(row)
            cb_t = const.tile([128, NCB], F32, tag="convb")
            nc.sync.dma_start(out=cb_t[:], in_=convb[:])
            wxp_t = []
            for kc in range(NCB):
                t = const.tile([128, 96], BF16, tag=f"wxp{kc}")
                nc.sync.dma_start(out=t[:], in_=wxpT[kc * 128:(kc + 1) * 128, :])
                wxp_t.append(t)

            # xi (post in_proj, pre conv): padded with 3 zero columns in front
            xi_pad = []
            for cb in range(NCB):
                t = const.tile([128, L + D_CONV - 1], BF16, tag=f"xip{cb}")
                nc.vector.memset(t[:, 0:D_CONV - 1], 0.0)
                xi_pad.append(t)
            xc_t = [const.tile([128, L], BF16, tag=f"xc{cb}", name=f"xc{cb}") for cb in range(NCB)]

            # ---- per token-block: in_proj -> conv -> x_proj, so the dbc
            # partials stream out as soon as each block is done
            def a_inproj(n, m):
                pt = psum.tile([128, 512], F32, tag="mm")
                for k in range(KM):
                    nc.tensor.matmul(
                        pt[:], w1_t[k][:, m * 128:(m + 1) * 128],
                        xT_t[k][:, n * 512:(n + 1) * 512],
                        start=(k == 0), stop=(k == KM - 1))
                if m < NCB:
                    nc.scalar.activation(
                        xi_pad[m][:, D_CONV - 1 + n * 512:
                                  D_CONV - 1 + (n + 1) * 512],
                        pt[:], AF.Copy)
                else:
                    st = work.tile([128, 512], BF16, tag="sres")
                    nc.scalar.activation(st[:], pt[:], AF.Silu)
                    for j in range(2):
                        nc.sync.dma_start(
                            out=sres_out[(m - NCB) * 128:(m - NCB + 1) * 128,
                                         n * 512 + j * 256:
                                         n * 512 + (j + 1) * 256],
                            in_=st[:, j * 256:(j + 1) * 256])

            def a_conv(n, cb):
                pt = psum.tile([128, 512], F32, tag="mm")
                for tap in range(D_CONV):
                    nc.tensor.matmul(
                        pt[:], cdiag[tap][cb][:],
                        xi_pad[cb][:, n * 512 + tap:n * 512 + tap + 512],
                        start=(tap == 0), stop=(tap == D_CONV - 1))
                nc.scalar.activation(
                    xc_t[cb][:, n * 512:(n + 1) * 512], pt[:], AF.Silu,
                    bias=cb_t[:, cb:cb + 1])
                for j in range(2):
                    sl = slice(n * 512 + j * 256, n * 512 + (j + 1) * 256)
                    nc.sync.dma_start(out=xc_out[cb * 128:(cb + 1) * 128, sl],
                                      in_=xc_t[cb][:, sl])

            for n in range(NT):
                # in_proj xi half, conv, in_proj res half, x_proj.  On the
                # last block the convs run before the res half so the xc
                # and dbc writeouts aren't serialized behind it.
                for m in range(NCB):
                    a_inproj(n, m)
                if n == NT - 1:
                    for cb in range(NCB):
                        a_conv(n, cb)
                    for m in range(NCB, 2 * NCB):
                        a_inproj(n, m)
                else:
                    for m in range(NCB, 2 * NCB):
                        a_inproj(n, m)
                    for cb in range(NCB):
                        a_conv(n, cb)
                # x_proj partial: dbc = wxpT.T @ xc   [96, 512]
                pt = psum.tile([96, 512], F32, tag="xp")
                for kc in range(NCB):
                    nc.tensor.matmul(
                        pt[:], wxp_t[kc][:],
                        xc_t[kc][:, n * 512:(n + 1) * 512],
                        start=(kc == 0), stop=(kc == NCB - 1))
                dt = work.tile([96, 512], F32, tag="dbc")
                nc.scalar.activation(dt[:], pt[:], AF.Copy)
                for j in range(4):
                    nc.sync.dma_start(
                        out=dbc_out[:, n * 512 + j * 128:n * 512 + (j + 1) * 128],
                        in_=dt[:, j * 128:(j + 1) * 128])
    nc.compile()
    return nc


def _build_b():
    nc = bacc.Bacc("TRN2", target_bir_lowering=False, debug=False,
                   num_devices=NCORES)
    xc_in = nc.dram_tensor("xc", [CH, L], BF16, kind="ExternalInput").ap()
    sres_in = nc.dram_tensor("sres", [CH, L], BF16, kind="ExternalInput").ap()
    dtr = nc.dram_tensor("dtr", [DT_RANK, L], F32, kind="ExternalInput").ap()
    brep = nc.dram_tensor("brep", [D_STATE * 128, L], BF16,
                          kind="ExternalInput").ap()
    crep = nc.dram_tensor("crep", [D_STATE * 128, L], BF16,
                          kind="ExternalInput").ap()
    wdtT = nc.dram_tensor("wdtT", [DT_RANK, CH], F32, kind="ExternalInput").ap()
    dtb = nc.dram_tensor("dtb", [128, NCB], F32, kind="ExternalInput").ap()
    woutT = nc.dram_tensor("woutT", [CH, D_MODEL], BF16,
                           kind="ExternalInput").ap()
    acol = nc.dram_tensor("acol", [128, D_STATE * NCB], F32,
                          kind="ExternalInput").ap()
    dcol = nc.dram_tensor("dcol", [128, NCB], F32, kind="ExternalInput").ap()
    ident = nc.dram_tensor("ident", [128, 128], BF16, kind="ExternalInput").ap()

    outp = nc.dram_tensor("outp", [NCB * D_MODEL, L], BF16,
                          kind="ExternalOutput").ap()

    with TileContext(nc) as tc:
        with (
            tc.tile_pool(name="const", bufs=1) as const,
            tc.tile_pool(name="bc", bufs=5) as bcpool,
            tc.tile_pool(name="dap", bufs=4) as dapool,
            tc.tile_pool(name="work", bufs=3) as work,
        ):
            # small tiles first: the dt_proj/delta path gates the first scan.
            # Early-needed big tiles are split into column chunks so they
            # spread across DMA queues (~15GB/s per queue).
            dtr_t = const.tile([DT_RANK, L], F32, tag="dtr")
            wdt_t = const.tile([DT_RANK, CH], F32, tag="wdt")
            for n in range(2):
                nc.sync.dma_start(out=dtr_t[:, n * 256:(n + 1) * 256],
                                  in_=dtr[:, n * 256:(n + 1) * 256])
            for j in range(2):
                nc.sync.dma_start(out=wdt_t[:, j * 256:(j + 1) * 256],
                                  in_=wdtT[:, j * 256:(j + 1) * 256])
            for n in range(2, 8):
                nc.sync.dma_start(out=dtr_t[:, n * 256:(n + 1) * 256],
                                  in_=dtr[:, n * 256:(n + 1) * 256])
            dtb_t = const.tile([128, NCB], F32, tag="dtb")
            nc.sync.dma_start(out=dtb_t[:], in_=dtb[:])
            acol_t = const.tile([128, D_STATE * NCB], F32, tag="acol")
            nc.sync.dma_start(out=acol_t[:], in_=acol[:])
            dcol_t = const.tile([128, NCB], F32, tag="dcol")
            nc.sync.dma_start(out=dcol_t[:], in_=dcol[:])
            id_t = const.tile([128, 128], BF16, tag="ident")
            nc.sync.dma_start(out=id_t[:], in_=ident[:])
            xc_t, sres_t, wout_t = [], [], []
            for cb in range(NCB):
                t = const.tile([128, L], BF16, tag=f"xc{cb}", name=f"xc{cb}")
                xc_t.append(t)
            for q4 in range(4):
                nc.sync.dma_start(
                    out=xc_t[0][:, q4 * 512:(q4 + 1) * 512],
                    in_=xc_in[0:128, q4 * 512:(q4 + 1) * 512])
            # br/cr for the first states, ahead of the bulk const DMAs so
            # the first scans aren't queued behind 5MB of loads
            NPRE = 3
            bc_pre = []
            for s in range(NPRE):
                br = bcpool.tile([128, L], BF16, tag="brep", name=f"brp{s}")
                cr = bcpool.tile([128, L], BF16, tag="crep", name=f"crp{s}")
                nsplit = 2 if s < 2 else 1
                for j in range(nsplit):
                    w = L // nsplit
                    nc.sync.dma_start(
                        out=br[:, j * w:(j + 1) * w],
                        in_=brep[s * 128:(s + 1) * 128, j * w:(j + 1) * w])
                    nc.sync.dma_start(
                        out=cr[:, j * w:(j + 1) * w],
                        in_=crep[s * 128:(s + 1) * 128, j * w:(j + 1) * w])
                bc_pre.append((br, cr))
            for cb in range(1, NCB):
                nc.sync.dma_start(out=xc_t[cb][:],
                                  in_=xc_in[cb * 128:(cb + 1) * 128, :])

            # ---- dt_proj -> z = w.T@dtr + b evicted to SBUF f32 eagerly
            # (plain Copy, no ACT table), so psum1 frees early and the
            # softplus Exp/Ln pairs can be scheduled lazily per cb without
            # holding PSUM banks.
            z_t = []
            with tc.tile_pool(name="psum1", bufs=4, space="PSUM") as psum1:
                # m=0: softplus straight off PSUM (shortest path to the
                # first scan); m=1..3: evict z to SBUF, softplus later
                zt = const.tile([128, L], F32, tag="dl0", name="dl0")
                ets0 = []
                for n in range(NT):
                    pt = psum1.tile([128, 512], F32, tag="mm")
                    nc.tensor.matmul(pt[:], wdt_t[:, 0:128],
                                     dtr_t[:, n * 512:(n + 1) * 512],
                                     start=True, stop=True)
                    et = work.tile([128, 512], F32, tag="spe",
                                   name=f"spe0_{n}")
                    nc.scalar.activation(et[:], pt[:], AF.Exp,
                                         bias=dtb_t[:, 0:1])
                    ets0.append(et)
                for n in range(NT):
                    nc.scalar.activation(zt[:, n * 512:(n + 1) * 512],
                                         ets0[n][:], AF.Ln, bias=1.0)
                z_t.append(zt)
                for cb_ in range(NCB):
                    t = const.tile([128, L], BF16, tag=f"sr{cb_}",
                                   name=f"sr{cb_}")
                    nc.sync.dma_start(
                        out=t[:], in_=sres_in[cb_ * 128:(cb_ + 1) * 128, :])
                    sres_t.append(t)
                for kc in range(NCB):
                    t = const.tile([128, D_MODEL], BF16, tag=f"wo{kc}",
                                   name=f"wo{kc}")
                    nc.sync.dma_start(
                        out=t[:], in_=woutT[kc * 128:(kc + 1) * 128, :])
                    wout_t.append(t)
                # pre-emit the first dA exps for cb0 so they sit ahead of
                # the m1-3 eviction copies in the ACT queue
                da_pre = []
                dA0 = dapool.tile([128, L], BF16, tag="dA", name="dAp0")
                for q in range(NT):
                    nc.scalar.activation(dA0[:, q * 512:(q + 1) * 512],
                                         zt[:, q * 512:(q + 1) * 512], AF.Exp,
                                         scale=acol_t[:, 0:1])
                da_pre.append(dA0)
                for s in range(1, 3):
                    dA = dapool.tile([128, L], BF16, tag="dA", name=f"dAp{s}")
                    nc.scalar.activation(dA[:], zt[:], AF.Exp,
                                         scale=acol_t[:, s * NCB:s * NCB + 1])
                    da_pre.append(dA)
                for m in range(1, NCB):
                    zt = const.tile([128, L], F32, tag=f"dl{m}", name=f"dl{m}")
                    for n in range(NT):
                        pt = psum1.tile([128, 512], F32, tag="mm")
                        nc.tensor.matmul(pt[:], wdt_t[:, m * 128:(m + 1) * 128],
                                         dtr_t[:, n * 512:(n + 1) * 512],
                                         start=True, stop=True)
                        nc.scalar.activation(zt[:, n * 512:(n + 1) * 512],
                                             pt[:], AF.Copy)
                    z_t.append(zt)

            def softplus_m(m):
                # delta = ln(exp(z + b) + 1), batched Exps then Lns to
                # limit ACT-table swaps; overwrites z_t[m] in place
                ets = []
                for n in range(NT):
                    et = work.tile([128, 512], F32, tag="spe",
                                   name=f"spe{m}_{n}")
                    nc.scalar.activation(et[:], z_t[m][:, n * 512:(n + 1) * 512],
                                         AF.Exp, bias=dtb_t[:, m:m + 1])
                    ets.append(et)
                for n in range(NT):
                    nc.scalar.activation(z_t[m][:, n * 512:(n + 1) * 512],
                                         ets[n][:], AF.Ln, bias=1.0)

            delta_t = z_t

            # u = delta * xc tiles, filled lazily at the start of each cb pass
            u_t = [const.tile([128, L], BF16, tag=f"u{cb}", name=f"u{cb}")
                   for cb in range(NCB)]

            # ---- the scan, one channel-block pass at a time: the 16
            # C-weighted state contributions are summed on the PE via
            # identity-matmul accumulation into one [128,L] fp32 PSUM
            # accumulator (4 banks).  The other 4 banks host the per-cb
            # out_proj partial right after each pass, so the out_proj
            # work overlaps the next pass's scans instead of being a
            # serial tail.  The host sums the per-cb output partials.
            with tc.tile_pool(name="psum2", bufs=2, space="PSUM") as psum2:
              for cb in range(NCB):
                with tc.tile_pool(name=f"accp{cb}", bufs=1,
                                  space="PSUM") as accpool:
                    accp = accpool.tile([128, L], F32, tag="ac", name="accp")
                    if cb == 0:
                        for q in range(NT):
                            qs = slice(q * 512, (q + 1) * 512)
                            nc.vector.tensor_mul(u_t[cb][:, qs],
                                                 delta_t[cb][:, qs],
                                                 xc_t[cb][:, qs])
                    else:
                        nc.vector.tensor_mul(u_t[cb][:], delta_t[cb][:],
                                             xc_t[cb][:])
                    for s in range(D_STATE):
                        if cb == 0 and s < NPRE:
                            br, cr = bc_pre[s]
                        else:
                            br = bcpool.tile([128, L], BF16, tag="brep")
                            nc.sync.dma_start(
                                out=br[:], in_=brep[s * 128:(s + 1) * 128, :])
                            cr = bcpool.tile([128, L], BF16, tag="crep")
                            nc.sync.dma_start(
                                out=cr[:], in_=crep[s * 128:(s + 1) * 128, :])
                        if cb == 0 and s < 3:
                            dA = da_pre[s]
                        else:
                            dA = dapool.tile([128, L], BF16, tag="dA")
                            nc.scalar.activation(dA[:], delta_t[cb][:], AF.Exp,
                                                 scale=acol_t[:, s * NCB + cb:
                                                              s * NCB + cb + 1])
                        bu = work.tile([128, L], BF16, tag="bu")
                        h = work.tile([128, L], BF16, tag="h")
                        if cb == 0 and s == 0:
                            # chunked first scan: starts as soon as the
                            # first delta/dA chunk lands instead of the
                            # whole row; chained via initial=
                            for q in range(NT):
                                qs = slice(q * 512, (q + 1) * 512)
                                nc.vector.tensor_mul(bu[:, qs], u_t[cb][:, qs],
                                                     br[:, qs])
                                init = (0.0 if q == 0
                                        else h[:, q * 512 - 1:q * 512])
                                nc.vector.tensor_tensor_scan(
                                    h[:, qs], dA[:, qs], bu[:, qs], init,
                                    OP.mult, OP.add)
                        else:
                            nc.vector.tensor_mul(bu[:], u_t[cb][:], br[:])
                            nc.vector.tensor_tensor_scan(h[:], dA[:], bu[:],
                                                         0.0, OP.mult, OP.add)
                        hc = work.tile([128, L], BF16, tag="hc")
                        nc.vector.tensor_mul(hc[:], h[:], cr[:])
                        for n in range(NT):
                            nc.tensor.matmul(
                                accp[:, n * 512:(n + 1) * 512],
                                id_t[:],
                                hc[:, n * 512:(n + 1) * 512],
                                start=(s == 0), stop=(s == D_STATE - 1))
                        if s == 6 and cb + 1 < NCB:
                            # schedule the next block's softplus mid-pass:
                            # ACT has slack here, and this keeps it off the
                            # ramp and off the next pass's critical path
                            softplus_m(cb + 1)
                    # ---- y = (acc + xc * D) * sres; y overwrites the spent
                    # xc tile (WAR handled by tile dep tracking)
                    for n in range(NT):
                        sl = slice(n * 512, (n + 1) * 512)
                        t1 = work.tile([128, 512], BF16, tag="t1")
                        nc.vector.scalar_tensor_tensor(
                            t1[:], xc_t[cb][:, sl], dcol_t[:, cb:cb + 1],
                            accp[:, sl], OP.mult, OP.add)
                        nc.vector.tensor_mul(xc_t[cb][:, sl], t1[:],
                                             sres_t[cb][:, sl])
                # ---- out_proj partial for this cb: woutT[cb].T @ y[cb].
                # The last block's eviction runs on the (now idle) DVE; the
                # earlier blocks' evictions run on ACT under the scans.
                for nh in range(2):
                    for m in range(D_MODEL // 128):
                        sl = slice(nh * 1024, (nh + 1) * 1024)
                        pt = psum2.tile([128, 1024], F32, tag="mm")
                        for q in range(2):
                            nc.tensor.matmul(
                                pt[:, q * 512:(q + 1) * 512],
                                wout_t[cb][:, m * 128:(m + 1) * 128],
                                xc_t[cb][:, nh * 1024 + q * 512:
                                          nh * 1024 + (q + 1) * 512],
                                start=True, stop=True)
                        ot = work.tile([128, 1024], BF16, tag="ot")
                        if cb == NCB - 1 and (nh * 8 + m) % 8 < 3:
                            nc.vector.tensor_copy(ot[:], pt[:])
                        else:
                            nc.scalar.activation(ot[:], pt[:], AF.Copy)
                        nsp = 2 if cb == NCB - 1 else 1
                        for j in range(nsp):
                            w = 1024 // nsp
                            nc.sync.dma_start(
                                out=outp[cb * D_MODEL + m * 128:
                                         cb * D_MODEL + (m + 1) * 128,
                                         nh * 1024 + j * w:
                                         nh * 1024 + (j + 1) * w],
                                in_=ot[:, j * w:(j + 1) * w])
    nc.compile()
    return nc


def _bf(a):
    return np.ascontiguousarray(a).astype(ml_dtypes.bfloat16)


def _f32(a):
    return np.ascontiguousarray(a, dtype=np.float32)


def kernel(x, in_proj_w, conv_w, conv_b, x_proj_w, dt_proj_w, dt_proj_b,
           A_log, D, out_proj_w):
    if "a" not in _CACHE:
        _CACHE["a"] = _build_a()
    if "b" not in _CACHE:
        _CACHE["b"] = _build_b()
    nca, ncb = _CACHE["a"], _CACHE["b"]

    A = -np.exp(np.asarray(A_log, np.float32))          # [D_INNER, D_STATE]
    x = np.asarray(x, np.float32)

    core_bq = [(c // 4, c % 4) for c in range(NCORES)]

    # ---------------- kernel A inputs
    xTb = [_bf(x[b].T) for b in range(B)]
    in_maps = []
    for b, q in core_bq:
        sl = slice(q * CH, (q + 1) * CH)
        w1 = np.concatenate([in_proj_w[sl], in_proj_w[D_INNER + q * CH:
                                                      D_INNER + (q + 1) * CH]], 0)
        cw = conv_w[sl, 0, :]                            # [CH, 4]
        cd = np.zeros((D_CONV * NCB * 128, 128), np.float32)
        for tap in range(D_CONV):
            for cb in range(NCB):
                blk = cd[(tap * NCB + cb) * 128:(tap * NCB + cb + 1) * 128]
                np.fill_diagonal(blk, cw[cb * 128:(cb + 1) * 128, tap])
        in_maps.append({
            "xT": xTb[b],
            "w1t": _bf(w1.T),
            "convdiag": _bf(cd),
            "convb": _f32(conv_b[sl].reshape(NCB, 128).T),
            "wxpT": _bf(x_proj_w[:, sl].T),
        })
    ra = run_bass_kernel_spmd(nca, in_maps, list(range(NCORES)))

    # ---------------- host exchange
    dbc = [None, None]
    for b in range(B):
        dbc[b] = sum(np.asarray(ra.results[4 * b + q]["dbc"], np.float32)
                     for q in range(4))
    in_maps_b = []
    breps, creps = [], []
    for b in range(B):
        Bm = dbc[b][DT_RANK:DT_RANK + D_STATE]           # [16, L]
        Cm = dbc[b][DT_RANK + D_STATE:]
        breps.append(_bf(np.repeat(Bm, 128, axis=0)))
        creps.append(_bf(np.repeat(Cm, 128, axis=0)))
    for c, (b, q) in enumerate(core_bq):
        sl = slice(q * CH, (q + 1) * CH)
        acol = np.zeros((128, D_STATE * NCB), np.float32)
        for s in range(D_STATE):
            for cb in range(NCB):
                acol[:, s * NCB + cb] = A[q * CH + cb * 128:
                                          q * CH + (cb + 1) * 128, s]
        in_maps_b.append({
            "xc": ra.results[c]["xc"],
            "sres": ra.results[c]["sres"],
            "dtr": _f32(dbc[b][:DT_RANK]),
            "brep": breps[b],
            "crep": creps[b],
            "wdtT": _f32(dt_proj_w[sl].T),
            "dtb": _f32(dt_proj_b[sl].reshape(NCB, 128).T),
            "woutT": _bf(out_proj_w[:, sl].T),
            "acol": acol,
            "dcol": _f32(D[sl].reshape(NCB, 128).T),
            "ident": _bf(np.eye(128, dtype=np.float32)),
        })
    rb = run_bass_kernel_spmd(ncb, in_maps_b, list(range(NCORES)))

    out = np.zeros((B, L, D_MODEL), np.float32)
    for b in range(B):
        acc = sum(np.asarray(rb.results[4 * b + q]["outp"], np.float32)
                  .reshape(NCB, D_MODEL, L).sum(0)
                  for q in range(4))
        out[b] = acc.T
    return out

